# revision 11
# baseline (speedup 1.0000x reference)
"""Multi-head attention on 8 Trainium2 NeuronCores.

Problem shape: x[4, 2048, 1024], H=16 heads, Dh=64, fp32.
Sharding: core c handles batch b = c//2 and heads 8*(c%2) .. 8*(c%2)+8.
Each core computes its 8 heads' attention + the partial W_O contraction
for its batch; the host sums the two half-head partials per batch and
adds b_O (plus the b_V @ W_O constant row, folded host-side since
softmax rows sum to 1).  No collectives needed.

All matmul operands are bf16 (fp32 psum accumulation); bf16 enables the
compiler's fast-weight-load path so LDWEIGHTS hides behind the matmul
stream.  ACT exp (33.5M elems/core, ~280us busy) is the floor; the
emission schedule keeps it dense:
  - x^T resident in SBUF (8 tiles [128, 2048]), loaded once.
  - K^T chunks and Q^T(pair0) emitted just-in-time so the first exp
    fires a few us in; V tiles interleave with pair0's score groups;
    K^T chunks for pair j are built inside pairs j-1/j; Q^T for
    (pair j+1, qc) hides inside (pair j, qc).
  - per sp: scores (2 s-tiles x 2 row-tiled head halves) -> exp
    [128,1024] per half -> lag-1 AV (M=65 with ones denom column).
Device-side layout per core (host pre-transposed):
  xT   [1024, 2048]  = x[b].T                                 [d, t]
  wqT/wkT/wvT [1024, 512] = W[heads].reshape(512,1024).T      [d, (h,k)]
  woT  [512, 1024]   = W_O[heads].transpose(0,2,1).reshape    [(h,k), d]
  bq/bk [128, 4]     per-partition bias layout (col m = (h,k) m*128..)
Output: out [2048, 1024] partial (pre-bias) for this core's batch.
"""

import numpy as np
from contextlib import ExitStack

import ml_dtypes

import concourse.bass as bass
import concourse.mybir as mybir
import concourse.tile as tile
from concourse import bacc
from concourse.bass_utils import run_bass_kernel_spmd

F32 = mybir.dt.float32
BF16 = mybir.dt.bfloat16
AF = mybir.ActivationFunctionType

T = 2048          # tokens
D = 1024          # d_model
HK = 512          # 8 local heads x 64
NH = 8            # local heads
DH = 64           # head dim
NDT = 8           # d-tiles of 128
NTT = 16          # t-tiles of 128
NMT = 4           # (h,k) m-tiles of 128 == head pairs
NQC = 4           # q-chunks of 512
NST = 16          # s-tiles of 128
VW = NH * (DH + 1)  # V_aug width: 8 heads x (64 + ones col)


def build():
    nc = bacc.Bacc("TRN2", target_bir_lowering=False, debug=False)

    xT_d = nc.dram_tensor("xT", [D, T], BF16, kind="ExternalInput").ap()
    wq_d = nc.dram_tensor("wqT", [D, HK], BF16, kind="ExternalInput").ap()
    wk_d = nc.dram_tensor("wkT", [D, HK], BF16, kind="ExternalInput").ap()
    wv_d = nc.dram_tensor("wvT", [D, HK], BF16, kind="ExternalInput").ap()
    wo_d = nc.dram_tensor("woT", [HK, D], BF16, kind="ExternalInput").ap()
    bq_d = nc.dram_tensor("bq", [128, 4], F32, kind="ExternalInput").ap()
    bk_d = nc.dram_tensor("bk", [128, 4], F32, kind="ExternalInput").ap()
    ones_d = nc.dram_tensor("ones", [128, DH], BF16, kind="ExternalInput").ap()
    out_d = nc.dram_tensor("out", [T, D], F32, kind="ExternalOutput").ap()

    with tile.TileContext(nc) as tc, ExitStack() as ctx:
        # ---------------- SBUF pools ----------------
        const = ctx.enter_context(tc.tile_pool(name="const", bufs=1))
        bq_sb = const.tile([128, 4], F32, tag="bq", name="bq")
        bk_sb = const.tile([128, 4], F32, tag="bk", name="bk")
        ones_sb = const.tile([128, DH], BF16, tag="ones", name="ones")

        persist = ctx.enter_context(tc.tile_pool(name="persist", bufs=1))
        # x resident as chunk-major [d-tile][chunk] so chunk c's consumers
        # only wait on chunk c's DMAs
        xsb = [[persist.tile([128, 512], BF16, tag=f"x{i}_{c}",
                             name=f"x{i}_{c}") for c in range(4)]
               for i in range(NDT)]
        KT = [persist.tile([128, T], BF16, tag=f"kt{m}", name=f"kt{m}")
              for m in range(NMT)]
        V = [persist.tile([128, VW], BF16, tag=f"v{t}", name=f"v{t}")
             for t in range(NTT)]

        wpool = ctx.enter_context(tc.tile_pool(name="wpool", bufs=1))
        wk_sb = [wpool.tile([128, HK], BF16, tag=f"wk{i}", name=f"wk{i}")
                 for i in range(NDT)]
        wq_sb = [wpool.tile([128, HK], BF16, tag=f"wq{i}", name=f"wq{i}")
                 for i in range(NDT)]
        wv_sb = [wpool.tile([128, HK], BF16, tag=f"wv{i}", name=f"wv{i}")
                 for i in range(NDT)]
        wo_sb = [wpool.tile([128, D], BF16, tag=f"wo{jj}", name=f"wo{jj}")
                 for jj in range(NMT)]

        qtpool = ctx.enter_context(tc.tile_pool(name="qtpool", bufs=2))
        epool = ctx.enter_context(tc.tile_pool(name="epool", bufs=1))
        otpool = ctx.enter_context(tc.tile_pool(name="otpool", bufs=1))
        opool = ctx.enter_context(tc.tile_pool(name="opool", bufs=2))
        foutp = ctx.enter_context(tc.tile_pool(name="foutp", bufs=2))

        # ---------------- PSUM pools: 4 + 2 + 2 = 8 banks -----------------
        scps = ctx.enter_context(
            tc.tile_pool(name="sc_ps", bufs=2, space="PSUM"))
        avps = ctx.enter_context(
            tc.tile_pool(name="av_ps", bufs=2, space="PSUM"))
        fps = ctx.enter_context(
            tc.tile_pool(name="fps", bufs=2, space="PSUM"))

        # ---------------- input DMAs (wk + x first: KT0 path) -------------
        for i in range(NDT):
            nc.scalar.dma_start(wk_sb[i][:], wk_d[i * 128:(i + 1) * 128, :])
        for c in range(4):
            for i in range(NDT):
                nc.sync.dma_start(
                    xsb[i][c][:],
                    xT_d[i * 128:(i + 1) * 128, c * 512:(c + 1) * 512])
        for i in range(NDT):
            nc.scalar.dma_start(wq_sb[i][:], wq_d[i * 128:(i + 1) * 128, :])
        for i in range(NDT):
            nc.scalar.dma_start(wv_sb[i][:], wv_d[i * 128:(i + 1) * 128, :])
        for jj in range(NMT):
            nc.scalar.dma_start(wo_sb[jj][:], wo_d[jj * 128:(jj + 1) * 128, :])
        nc.sync.dma_start(bq_sb[:], bq_d)
        nc.sync.dma_start(bk_sb[:], bk_d)
        nc.sync.dma_start(ones_sb[:], ones_d)

        # ---------------- projection emitters (all via fps slot) ----------
        def kt_chunk(m, c):
            csl = slice(c * 512, (c + 1) * 512)
            msl = slice(m * 128, (m + 1) * 128)
            ps = fps.tile([128, 512], F32, tag="fp", name="ktp")
            for i in range(NDT):
                nc.tensor.matmul(ps[:], wk_sb[i][:, msl], xsb[i][c][:],
                                 start=(i == 0), stop=(i == NDT - 1))
            nc.vector.tensor_scalar_add(KT[m][:, csl], ps[:],
                                        bk_sb[:, m:m + 1])

        QT = {}  # (j, qc) -> tile

        def qt_chunk(j, qc):
            msl = slice(j * 128, (j + 1) * 128)
            qt = qtpool.tile([128, 512], BF16, tag=f"qt{j}", name=f"qt{j}")
            ps = fps.tile([128, 512], F32, tag="fp", name="qtp")
            for i in range(NDT):
                nc.tensor.matmul(ps[:], wq_sb[i][:, msl], xsb[i][qc][:],
                                 start=(i == 0), stop=(i == NDT - 1))
            nc.vector.tensor_scalar_add(qt[:], ps[:], bq_sb[:, j:j + 1])
            QT[(j, qc)] = qt

        def v_tile(t_idx):
            c, vt = t_idx // 4, t_idx % 4
            vsl = slice(vt * 128, (vt + 1) * 128)
            ps = fps.tile([128, 512], F32, tag="fp", name="vps")
            for i in range(NDT):
                nc.tensor.matmul(ps[:], xsb[i][c][:, vsl], wv_sb[i][:],
                                 start=(i == 0), stop=(i == NDT - 1))
            v3 = V[t_idx][:].rearrange("p (h c) -> p h c", c=DH + 1)
            nc.vector.tensor_copy(
                v3[:, :, 0:DH],
                ps[:].rearrange("p (h c) -> p h c", c=DH))
            nc.vector.tensor_copy(
                v3[:, :, DH:DH + 1],
                ones_sb[:, 0:NH].rearrange("p (h o) -> p h o", o=1))

        def o_group(OT, qc, g):
            tt, dc = g // 2, g % 2
            tq = qc * 512 + tt * 128
            dsl = slice(dc * 512, (dc + 1) * 512)
            ps = fps.tile([128, 512], F32, tag="fp", name="fp")
            for jj in range(NMT):
                nc.tensor.matmul(ps[:],
                                 OT[jj][:, tt * 128:(tt + 1) * 128],
                                 wo_sb[jj][:, dsl],
                                 start=(jj == 0), stop=(jj == NMT - 1))
            ob = foutp.tile([128, 512], F32, tag="ob", name="ob")
            nc.vector.tensor_copy(ob[:], ps[:])
            nc.sync.dma_start(out_d[tq:tq + 128, dsl], ob[:])

        # ---------------- attention pair machinery ----------------
        def make_pair(j, QTj, OT):
            avp = {}
            for hl in (0, 1):
                avp[hl] = avps.tile([DH + 1, 512], F32,
                                    tag="av", name=f"av{hl}")
            state = {"pending": []}

            def emit_av(es_prev, sp_prev):
                for hl in (0, 1):
                    h = 2 * j + hl
                    for k in (0, 1):
                        st = 2 * sp_prev + k
                        nc.tensor.matmul(
                            avp[hl][:],
                            V[st][:, h * 65:h * 65 + 65],
                            es_prev[hl][:, k * 512:(k + 1) * 512],
                            start=(st == 0), stop=(st == NST - 1))

            def flush_av():
                for es_prev, sp_prev in state["pending"]:
                    emit_av(es_prev, sp_prev)
                state["pending"] = []

            def emit_sp(sp):
                if len(state["pending"]) == 2:
                    flush_av()
                sc = {}
                for hl in (0, 1):
                    sc[hl] = scps.tile([128, 1024], F32, tag="sc", name="sc")
                # prefetch both row-tile weight halves ahead of the MMs
                for k in (0, 1):
                    st = 2 * sp + k
                    ssl = slice(st * 128, (st + 1) * 128)
                    for hl in (0, 1):
                        psl = slice(hl * 64, (hl + 1) * 64)
                        nc.tensor.ldweights(KT[j][psl, ssl],
                                            tile_position=(hl * 64, 0))
                    for hl in (0, 1):
                        psl = slice(hl * 64, (hl + 1) * 64)
                        nc.tensor.matmul(
                            sc[hl][:, k * 512:(k + 1) * 512],
                            KT[j][psl, ssl], QTj[psl, :],
                            tile_position=(hl * 64, 0))
                es = {}
                for hl in (0, 1):
                    e = epool.tile([128, 1024], BF16,
                                   tag=f"e{hl}_{sp % 4}",
                                   name=f"e{hl}_{sp % 4}")
                    nc.scalar.activation(e[:], sc[hl][:], AF.Exp,
                                         scale=0.125)
                    es[hl] = e
                state["pending"].append((es, sp))

            def finalize():
                flush_av()
                avs, dn4, rc4, rcp, bcs = {}, {}, {}, {}, {}
                for hl in (0, 1):
                    avs[hl] = opool.tile([DH + 1, 512], F32, tag="avs",
                                         name="avs")
                    nc.vector.tensor_copy(avs[hl][:], avp[hl][:])
                for hl in (0, 1):
                    dn4[hl] = opool.tile([128, 4], F32, tag="dn4",
                                         name="dn4")
                    nc.sync.dma_start(dn4[hl][:], avs[hl][DH:DH + 1, :])
                for hl in (0, 1):
                    rc4[hl] = opool.tile([128, 4], F32, tag="rc4",
                                         name="rc4")
                    nc.vector.reciprocal(rc4[hl][:], dn4[hl][:])
                for hl in (0, 1):
                    rcp[hl] = opool.tile([1, 512], F32, tag="rcp",
                                         name="rcp")
                    nc.sync.dma_start(rcp[hl][:], rc4[hl][:])
                for hl in (0, 1):
                    bcs[hl] = opool.tile([DH, 512], F32, tag="bcs",
                                         name="bcs")
                    nc.gpsimd.partition_broadcast(bcs[hl][:], rcp[hl][:])
                for hl in (0, 1):
                    nc.vector.tensor_mul(OT[j][hl * 64:(hl + 1) * 64, :],
                                         avs[hl][0:DH, :], bcs[hl][:])

            return emit_sp, finalize

        # ---------------- emission schedule ----------------
        # head: just enough for pair0/qc0's first score group
        kt_chunk(0, 0)
        qt_chunk(0, 0)

        for qc in range(NQC):
            OT = [otpool.tile([128, 512], BF16, tag=f"ot{jj}",
                              name=f"ot{jj}") for jj in range(NMT)]
            for j in range(NMT):
                ex = {sp: [] for sp in range(8)}
                if qc == 0:
                    if j == 0:
                        # V in 4-tile bursts; KT0 tail chunks just-in-time
                        ex[0] += [(lambda t: lambda: v_tile(t))(t)
                                  for t in range(0, 4)]
                        ex[1] += [lambda: kt_chunk(0, 1)]
                        ex[2] += [(lambda t: lambda: v_tile(t))(t)
                                  for t in range(4, 8)]
                        ex[3] += [lambda: kt_chunk(0, 2)]
                        ex[4] += [(lambda t: lambda: v_tile(t))(t)
                                  for t in range(8, 12)]
                        ex[5] += [lambda: kt_chunk(0, 3)]
                        ex[6] += [(lambda t: lambda: v_tile(t))(t)
                                  for t in range(12, 16)]
                    if j < NMT - 1:
                        # build next pair's full KT + its Q^T inside this one
                        jn = j + 1
                        ex[1 if j == 0 else 1].append(
                            (lambda m: lambda: kt_chunk(m, 0))(jn))
                        ex[3].append(
                            (lambda m: lambda: kt_chunk(m, 1))(jn))
                        ex[5].append(
                            (lambda m: lambda: kt_chunk(m, 2))(jn))
                        ex[6].append(
                            (lambda m: lambda: kt_chunk(m, 3))(jn))
                        ex[6].append(
                            (lambda jj: lambda: qt_chunk(jj, 0))(jn))
                else:
                    if j < NMT - 1:
                        ex[6].append(
                            (lambda jj, qq: lambda: qt_chunk(jj, qq))(
                                j + 1, qc))
                if j == NMT - 1 and qc + 1 < NQC:
                    ex[1].append(
                        (lambda qq: lambda: qt_chunk(0, qq))(qc + 1))

                if j == 0 and qc > 0:
                    # previous qc's output projection hides in pair0's
                    # ACT-bound window while its finalize chain drains
                    for g in range(8):
                        ex[2 * (g // 2)].append(
                            (lambda o, q, gg: lambda: o_group(o, q, gg))(
                                OT_prev, qc - 1, g))
                emit_sp, finalize = make_pair(j, QT[(j, qc)], OT)
                for sp in range(8):
                    emit_sp(sp)
                    for th in ex[sp]:
                        th()
                finalize()
            OT_prev = OT
        for g in range(8):
            o_group(OT_prev, NQC - 1, g)

    nc.compile()
    return nc


_NC_CACHE = None


def _get_nc():
    global _NC_CACHE
    if _NC_CACHE is None:
        _NC_CACHE = build()
    return _NC_CACHE


def _bf16(x):
    return np.ascontiguousarray(x, dtype=np.float32).astype(ml_dtypes.bfloat16)


def _prep_core(x, W_Q, b_Q, W_K, b_K, W_V, b_V, W_O, core):
    b = core // 2
    hs = slice(8 * (core % 2), 8 * (core % 2) + 8)
    f32 = np.float32

    def bias_layout(bx):
        return np.ascontiguousarray(bx[hs].reshape(4, 128).T, dtype=f32)

    return {
        "xT": _bf16(x[b].T),
        "wqT": _bf16(W_Q[hs].reshape(HK, D).T),
        "wkT": _bf16(W_K[hs].reshape(HK, D).T),
        "wvT": _bf16(W_V[hs].reshape(HK, D).T),
        "woT": _bf16(W_O[hs].transpose(0, 2, 1).reshape(HK, D)),
        "bq": bias_layout(b_Q),
        "bk": bias_layout(b_K),
        "ones": np.ones((128, DH), dtype=ml_dtypes.bfloat16),
    }


def kernel(x, W_Q, b_Q, W_K, b_K, W_V, b_V, W_O, b_O, _trace=False):
    nc = _get_nc()
    in_maps = [
        _prep_core(x, W_Q, b_Q, W_K, b_K, W_V, b_V, W_O, c) for c in range(8)
    ]
    res = run_bass_kernel_spmd(nc, in_maps, core_ids=list(range(8)),
                               trace=_trace)
    out = np.empty((4, T, D), dtype=np.float32)
    for b in range(4):
        # b_V enters additively after softmax (rows sum to 1): fold
        # b_V @ W_O per half-head shard into the host-side bias.
        acc = res.results[2 * b]["out"].astype(np.float32).copy()
        acc += res.results[2 * b + 1]["out"]
        bias = b_O.astype(np.float64).copy()
        for c in (2 * b, 2 * b + 1):
            hs = slice(8 * (c % 2), 8 * (c % 2) + 8)
            bias += np.einsum("hk,hdk->d", b_V[hs].astype(np.float64),
                              W_O[hs].astype(np.float64))
        out[b] = acc + bias.astype(np.float32)[None, :]
    if _trace:
        kernel.last_results = res
    return out


# revision 12
# speedup vs baseline: 1.0317x; 1.0317x over previous
"""Multi-head attention on 8 Trainium2 NeuronCores.

Problem shape: x[4, 2048, 1024], H=16 heads, Dh=64, fp32.
Sharding: core c handles batch b = c//2 and heads 8*(c%2) .. 8*(c%2)+8.
Each core computes its 8 heads' attention + the partial W_O contraction
for its batch; the host sums the two half-head partials per batch and
adds b_O (plus the b_V @ W_O constant row, folded host-side since
softmax rows sum to 1).  No collectives needed.

All matmul operands are bf16 (fp32 psum accumulation); bf16 enables the
compiler's fast-weight-load path so LDWEIGHTS hides behind the matmul
stream.  ACT exp (33.5M elems/core, ~280us busy) is the floor; the
emission schedule keeps it dense:
  - x^T resident in SBUF (8 tiles [128, 2048]), loaded once.
  - K^T chunks and Q^T(pair0) emitted just-in-time so the first exp
    fires a few us in; V tiles interleave with pair0's score groups;
    K^T chunks for pair j are built inside pairs j-1/j; Q^T for
    (pair j+1, qc) hides inside (pair j, qc).
  - per sp: scores (2 s-tiles x 2 row-tiled head halves) -> exp
    [128,1024] per half -> lag-1 AV (M=65 with ones denom column).
Device-side layout per core (host pre-transposed):
  xT   [1024, 2048]  = x[b].T                                 [d, t]
  wqT/wkT/wvT [1024, 512] = W[heads].reshape(512,1024).T      [d, (h,k)]
  woT  [512, 1024]   = W_O[heads].transpose(0,2,1).reshape    [(h,k), d]
  bq/bk [128, 4]     per-partition bias layout (col m = (h,k) m*128..)
Output: out [2048, 1024] partial (pre-bias) for this core's batch.
"""

import numpy as np
from contextlib import ExitStack

import ml_dtypes

import concourse.bass as bass
import concourse.mybir as mybir
import concourse.tile as tile
from concourse import bacc
from concourse.bass_utils import run_bass_kernel_spmd

F32 = mybir.dt.float32
BF16 = mybir.dt.bfloat16
AF = mybir.ActivationFunctionType

T = 2048          # tokens
D = 1024          # d_model
HK = 512          # 8 local heads x 64
NH = 8            # local heads
DH = 64           # head dim
NDT = 8           # d-tiles of 128
NTT = 16          # t-tiles of 128
NMT = 4           # (h,k) m-tiles of 128 == head pairs
NQC = 4           # q-chunks of 512
NST = 16          # s-tiles of 128
VW = NH * (DH + 1)  # V_aug width: 8 heads x (64 + ones col)


def build():
    nc = bacc.Bacc("TRN2", target_bir_lowering=False, debug=False)

    xT_d = nc.dram_tensor("xT", [D, T], BF16, kind="ExternalInput").ap()
    wq_d = nc.dram_tensor("wqT", [D, HK], BF16, kind="ExternalInput").ap()
    wk_d = nc.dram_tensor("wkT", [D, HK], BF16, kind="ExternalInput").ap()
    wv_d = nc.dram_tensor("wvT", [D, HK], BF16, kind="ExternalInput").ap()
    wo_d = nc.dram_tensor("woT", [HK, D], BF16, kind="ExternalInput").ap()
    bq_d = nc.dram_tensor("bq", [128, 4], F32, kind="ExternalInput").ap()
    bk_d = nc.dram_tensor("bk", [128, 4], F32, kind="ExternalInput").ap()
    ones_d = nc.dram_tensor("ones", [128, DH], BF16, kind="ExternalInput").ap()
    out_d = nc.dram_tensor("out", [T, D], F32, kind="ExternalOutput").ap()

    with tile.TileContext(nc) as tc, ExitStack() as ctx:
        # ---------------- SBUF pools ----------------
        const = ctx.enter_context(tc.tile_pool(name="const", bufs=1))
        bq_sb = const.tile([128, 4], F32, tag="bq", name="bq")
        bk_sb = const.tile([128, 4], F32, tag="bk", name="bk")
        ones_sb = const.tile([128, DH], BF16, tag="ones", name="ones")

        persist = ctx.enter_context(tc.tile_pool(name="persist", bufs=1))
        # x resident as chunk-major [d-tile][chunk] so chunk c's consumers
        # only wait on chunk c's DMAs
        xsb = [[persist.tile([128, 512], BF16, tag=f"x{i}_{c}",
                             name=f"x{i}_{c}") for c in range(4)]
               for i in range(NDT)]
        KT = [persist.tile([128, T], BF16, tag=f"kt{m}", name=f"kt{m}")
              for m in range(NMT)]
        V = [persist.tile([128, VW], BF16, tag=f"v{t}", name=f"v{t}")
             for t in range(NTT)]

        wpool = ctx.enter_context(tc.tile_pool(name="wpool", bufs=1))
        wk_sb = [wpool.tile([128, HK], BF16, tag=f"wk{i}", name=f"wk{i}")
                 for i in range(NDT)]
        wq_sb = [wpool.tile([128, HK], BF16, tag=f"wq{i}", name=f"wq{i}")
                 for i in range(NDT)]
        wv_sb = [wpool.tile([128, HK], BF16, tag=f"wv{i}", name=f"wv{i}")
                 for i in range(NDT)]
        wo_sb = [wpool.tile([128, D], BF16, tag=f"wo{jj}", name=f"wo{jj}")
                 for jj in range(NMT)]

        qtpool = ctx.enter_context(tc.tile_pool(name="qtpool", bufs=2))
        epool = ctx.enter_context(tc.tile_pool(name="epool", bufs=1))
        otpool = ctx.enter_context(tc.tile_pool(name="otpool", bufs=1))
        opool = ctx.enter_context(tc.tile_pool(name="opool", bufs=2))
        foutp = ctx.enter_context(tc.tile_pool(name="foutp", bufs=2))

        # ---------------- PSUM pools: 4 + 2 + 2 = 8 banks -----------------
        scps = ctx.enter_context(
            tc.tile_pool(name="sc_ps", bufs=2, space="PSUM"))
        avps = ctx.enter_context(
            tc.tile_pool(name="av_ps", bufs=2, space="PSUM"))
        fps = ctx.enter_context(
            tc.tile_pool(name="fps", bufs=2, space="PSUM"))

        # ---------------- input DMAs (wk + x first: KT0 path) -------------
        def x_chunk_dma(c):
            for i in range(NDT):
                nc.sync.dma_start(
                    xsb[i][c][:],
                    xT_d[i * 128:(i + 1) * 128, c * 512:(c + 1) * 512])

        def wo_dma():
            for jj in range(NMT):
                nc.scalar.dma_start(wo_sb[jj][:],
                                    wo_d[jj * 128:(jj + 1) * 128, :])

        # staged issue: only the head's needs go first so the DMA engines'
        # FIFOs aren't clogged by later chunks (xc2/xc3/wo defer into the
        # pair0 emission stream)
        nc.sync.dma_start(bq_sb[:], bq_d)
        nc.sync.dma_start(bk_sb[:], bk_d)
        nc.sync.dma_start(ones_sb[:], ones_d)
        for i in range(NDT):
            nc.scalar.dma_start(wk_sb[i][:], wk_d[i * 128:(i + 1) * 128, :])
        x_chunk_dma(0)
        for i in range(NDT):
            nc.scalar.dma_start(wq_sb[i][:], wq_d[i * 128:(i + 1) * 128, :])
        for i in range(NDT):
            nc.scalar.dma_start(wv_sb[i][:], wv_d[i * 128:(i + 1) * 128, :])
        x_chunk_dma(1)

        # ---------------- projection emitters (all via fps slot) ----------
        def kt_chunk(m, c):
            csl = slice(c * 512, (c + 1) * 512)
            msl = slice(m * 128, (m + 1) * 128)
            ps = fps.tile([128, 512], F32, tag="fp", name="ktp")
            for i in range(NDT):
                nc.tensor.matmul(ps[:], wk_sb[i][:, msl], xsb[i][c][:],
                                 start=(i == 0), stop=(i == NDT - 1))
            nc.vector.tensor_scalar_add(KT[m][:, csl], ps[:],
                                        bk_sb[:, m:m + 1])

        QT = {}  # (j, qc) -> tile

        def qt_chunk(j, qc):
            msl = slice(j * 128, (j + 1) * 128)
            qt = qtpool.tile([128, 512], BF16, tag=f"qt{j}", name=f"qt{j}")
            ps = fps.tile([128, 512], F32, tag="fp", name="qtp")
            for i in range(NDT):
                nc.tensor.matmul(ps[:], wq_sb[i][:, msl], xsb[i][qc][:],
                                 start=(i == 0), stop=(i == NDT - 1))
            nc.vector.tensor_scalar_add(qt[:], ps[:], bq_sb[:, j:j + 1])
            QT[(j, qc)] = qt

        def v_tile(t_idx):
            c, vt = t_idx // 4, t_idx % 4
            vsl = slice(vt * 128, (vt + 1) * 128)
            ps = fps.tile([128, 512], F32, tag="fp", name="vps")
            for i in range(NDT):
                nc.tensor.matmul(ps[:], xsb[i][c][:, vsl], wv_sb[i][:],
                                 start=(i == 0), stop=(i == NDT - 1))
            v3 = V[t_idx][:].rearrange("p (h c) -> p h c", c=DH + 1)
            nc.vector.tensor_copy(
                v3[:, :, 0:DH],
                ps[:].rearrange("p (h c) -> p h c", c=DH))
            nc.vector.tensor_copy(
                v3[:, :, DH:DH + 1],
                ones_sb[:, 0:NH].rearrange("p (h o) -> p h o", o=1))

        def o_group(OT, qc, g):
            tt, dc = g // 2, g % 2
            tq = qc * 512 + tt * 128
            dsl = slice(dc * 512, (dc + 1) * 512)
            ps = fps.tile([128, 512], F32, tag="fp", name="fp")
            for jj in range(NMT):
                nc.tensor.matmul(ps[:],
                                 OT[jj][:, tt * 128:(tt + 1) * 128],
                                 wo_sb[jj][:, dsl],
                                 start=(jj == 0), stop=(jj == NMT - 1))
            ob = foutp.tile([128, 512], F32, tag="ob", name="ob")
            nc.vector.tensor_copy(ob[:], ps[:])
            nc.sync.dma_start(out_d[tq:tq + 128, dsl], ob[:])

        # ---------------- attention pair machinery ----------------
        def make_pair(j, QTj, OT):
            avp = {}
            for hl in (0, 1):
                avp[hl] = avps.tile([DH + 1, 512], F32,
                                    tag="av", name=f"av{hl}")
            state = {"pending": []}

            def emit_av(es_prev, sp_prev):
                for hl in (0, 1):
                    h = 2 * j + hl
                    for k in (0, 1):
                        st = 2 * sp_prev + k
                        nc.tensor.matmul(
                            avp[hl][:],
                            V[st][:, h * 65:h * 65 + 65],
                            es_prev[hl][:, k * 512:(k + 1) * 512],
                            start=(st == 0), stop=(st == NST - 1))

            def flush_av():
                for es_prev, sp_prev in state["pending"]:
                    emit_av(es_prev, sp_prev)
                state["pending"] = []

            def emit_sp(sp):
                if len(state["pending"]) == 2:
                    flush_av()
                sc = {}
                for hl in (0, 1):
                    sc[hl] = scps.tile([128, 1024], F32, tag="sc", name="sc")
                # prefetch both row-tile weight halves ahead of the MMs
                for k in (0, 1):
                    st = 2 * sp + k
                    ssl = slice(st * 128, (st + 1) * 128)
                    for hl in (0, 1):
                        psl = slice(hl * 64, (hl + 1) * 64)
                        nc.tensor.ldweights(KT[j][psl, ssl],
                                            tile_position=(hl * 64, 0))
                    for hl in (0, 1):
                        psl = slice(hl * 64, (hl + 1) * 64)
                        nc.tensor.matmul(
                            sc[hl][:, k * 512:(k + 1) * 512],
                            KT[j][psl, ssl], QTj[psl, :],
                            tile_position=(hl * 64, 0))
                es = {}
                for hl in (0, 1):
                    e = epool.tile([128, 1024], BF16,
                                   tag=f"e{hl}_{sp % 4}",
                                   name=f"e{hl}_{sp % 4}")
                    nc.scalar.activation(e[:], sc[hl][:], AF.Exp,
                                         scale=0.125)
                    es[hl] = e
                state["pending"].append((es, sp))

            def finalize():
                flush_av()
                avs, dn4, rc4, rcp, bcs = {}, {}, {}, {}, {}
                for hl in (0, 1):
                    avs[hl] = opool.tile([DH + 1, 512], F32, tag="avs",
                                         name="avs")
                    nc.vector.tensor_copy(avs[hl][:], avp[hl][:])
                for hl in (0, 1):
                    dn4[hl] = opool.tile([128, 4], F32, tag="dn4",
                                         name="dn4")
                    nc.sync.dma_start(dn4[hl][:], avs[hl][DH:DH + 1, :])
                for hl in (0, 1):
                    rc4[hl] = opool.tile([128, 4], F32, tag="rc4",
                                         name="rc4")
                    nc.vector.reciprocal(rc4[hl][:], dn4[hl][:])
                for hl in (0, 1):
                    rcp[hl] = opool.tile([1, 512], F32, tag="rcp",
                                         name="rcp")
                    nc.sync.dma_start(rcp[hl][:], rc4[hl][:])
                for hl in (0, 1):
                    bcs[hl] = opool.tile([DH, 512], F32, tag="bcs",
                                         name="bcs")
                    nc.gpsimd.partition_broadcast(bcs[hl][:], rcp[hl][:])
                for hl in (0, 1):
                    nc.vector.tensor_mul(OT[j][hl * 64:(hl + 1) * 64, :],
                                         avs[hl][0:DH, :], bcs[hl][:])

            return emit_sp, finalize

        # ---------------- emission schedule ----------------
        # head: just enough for pair0/qc0's first score group
        kt_chunk(0, 0)
        qt_chunk(0, 0)

        for qc in range(NQC):
            OT = [otpool.tile([128, 512], BF16, tag=f"ot{jj}",
                              name=f"ot{jj}") for jj in range(NMT)]
            for j in range(NMT):
                ex = {sp: [] for sp in range(8)}
                if qc == 0:
                    if j == 0:
                        # V in 4-tile bursts; KT0 tail chunks just-in-time
                        ex[0] += [lambda: x_chunk_dma(2)]
                        ex[0] += [(lambda t: lambda: v_tile(t))(t)
                                  for t in range(0, 4)]
                        ex[1] += [lambda: x_chunk_dma(3),
                                  lambda: kt_chunk(0, 1)]
                        ex[2] += [(lambda t: lambda: v_tile(t))(t)
                                  for t in range(4, 8)]
                        ex[3] += [lambda: kt_chunk(0, 2)]
                        ex[4] += [(lambda t: lambda: v_tile(t))(t)
                                  for t in range(8, 12)]
                        ex[5] += [lambda: wo_dma(), lambda: kt_chunk(0, 3)]
                        ex[6] += [(lambda t: lambda: v_tile(t))(t)
                                  for t in range(12, 16)]
                    if j < NMT - 1:
                        # build next pair's full KT + its Q^T inside this one
                        jn = j + 1
                        ex[1 if j == 0 else 1].append(
                            (lambda m: lambda: kt_chunk(m, 0))(jn))
                        ex[3].append(
                            (lambda m: lambda: kt_chunk(m, 1))(jn))
                        ex[5].append(
                            (lambda m: lambda: kt_chunk(m, 2))(jn))
                        ex[6].append(
                            (lambda m: lambda: kt_chunk(m, 3))(jn))
                        ex[6].append(
                            (lambda jj: lambda: qt_chunk(jj, 0))(jn))
                else:
                    if j < NMT - 1:
                        ex[6].append(
                            (lambda jj, qq: lambda: qt_chunk(jj, qq))(
                                j + 1, qc))
                if j == NMT - 1 and qc + 1 < NQC:
                    ex[1].append(
                        (lambda qq: lambda: qt_chunk(0, qq))(qc + 1))

                if j == 0 and qc > 0:
                    # previous qc's output projection hides in pair0's
                    # ACT-bound window while its finalize chain drains
                    for g in range(8):
                        ex[2 * (g // 2)].append(
                            (lambda o, q, gg: lambda: o_group(o, q, gg))(
                                OT_prev, qc - 1, g))
                emit_sp, finalize = make_pair(j, QT[(j, qc)], OT)
                for sp in range(8):
                    emit_sp(sp)
                    for th in ex[sp]:
                        th()
                finalize()
            OT_prev = OT
        for g in range(8):
            o_group(OT_prev, NQC - 1, g)

    nc.compile()
    return nc


_NC_CACHE = None


def _get_nc():
    global _NC_CACHE
    if _NC_CACHE is None:
        _NC_CACHE = build()
    return _NC_CACHE


def _bf16(x):
    return np.ascontiguousarray(x, dtype=np.float32).astype(ml_dtypes.bfloat16)


def _prep_core(x, W_Q, b_Q, W_K, b_K, W_V, b_V, W_O, core):
    b = core // 2
    hs = slice(8 * (core % 2), 8 * (core % 2) + 8)
    f32 = np.float32

    def bias_layout(bx):
        return np.ascontiguousarray(bx[hs].reshape(4, 128).T, dtype=f32)

    return {
        "xT": _bf16(x[b].T),
        "wqT": _bf16(W_Q[hs].reshape(HK, D).T),
        "wkT": _bf16(W_K[hs].reshape(HK, D).T),
        "wvT": _bf16(W_V[hs].reshape(HK, D).T),
        "woT": _bf16(W_O[hs].transpose(0, 2, 1).reshape(HK, D)),
        "bq": bias_layout(b_Q),
        "bk": bias_layout(b_K),
        "ones": np.ones((128, DH), dtype=ml_dtypes.bfloat16),
    }


def kernel(x, W_Q, b_Q, W_K, b_K, W_V, b_V, W_O, b_O, _trace=False):
    nc = _get_nc()
    in_maps = [
        _prep_core(x, W_Q, b_Q, W_K, b_K, W_V, b_V, W_O, c) for c in range(8)
    ]
    res = run_bass_kernel_spmd(nc, in_maps, core_ids=list(range(8)),
                               trace=_trace)
    out = np.empty((4, T, D), dtype=np.float32)
    for b in range(4):
        # b_V enters additively after softmax (rows sum to 1): fold
        # b_V @ W_O per half-head shard into the host-side bias.
        acc = res.results[2 * b]["out"].astype(np.float32).copy()
        acc += res.results[2 * b + 1]["out"]
        bias = b_O.astype(np.float64).copy()
        for c in (2 * b, 2 * b + 1):
            hs = slice(8 * (c % 2), 8 * (c % 2) + 8)
            bias += np.einsum("hk,hdk->d", b_V[hs].astype(np.float64),
                              W_O[hs].astype(np.float64))
        out[b] = acc + bias.astype(np.float32)[None, :]
    if _trace:
        kernel.last_results = res
    return out
